# revision 1
# baseline (speedup 1.0000x reference)
"""Trainium2 Bass kernel for nn_Att_0_layer2 (sparse_attention).

Math (per (b, n) pair):
  v = att1 @ obj_reps                      # [A,O]@[O,D] -- never materialized:
  vq@W1 = v@W1v + q@W1q  ==>  att1 @ (obj_reps @ W1v) + (q @ W1q)
  jointT = relu(objW.T @ att1.T + bias)    # [H, A], objW = obj@W1v, bias = q@W1q + b1
  logits = jointT.T @ W2 (/t folded into W2 host-side; b2 dropped: softmax-invariant)
  att2 = softmax(logits masked by tags>0)
  out = att2 @ att1                        # [O]

Sharding: pure data parallel, B=64 split 8 ways (8 b's per core).
All device compute in bf16 matmuls w/ fp32 PSUM accumulate; softmax in fp32.
DMA strategy: one big DMA per att1[b,n]; small tensors batched in setup DMAs.
Software pipeline: stage A (load..exp) of pair p is emitted before stage B
(normalize + final einsum) of pair p-1 so the PE never stalls on the
cross-engine softmax round-trip.
"""

import sys
import os
import numpy as np

sys.path.insert(0, "/opt/trn_rl_repo")

B, N, A, O, D, Q, H = 64, 4, 1024, 128, 256, 256, 128
NCORES = 8
BPC = B // NCORES   # batches per core
P = 128             # partitions
AC = A // P         # a-chunks per pair

TRACE = False
TRACE_KW = {}

_NC = None


def _build_nc():
    import concourse.bacc as bacc
    import concourse.mybir as mybir
    from concourse.tile import TileContext
    from concourse.masks import make_identity

    f32 = mybir.dt.float32
    bf16 = mybir.dt.bfloat16
    i32 = mybir.dt.int32
    AF = mybir.ActivationFunctionType
    OP = mybir.AluOpType

    nc = bacc.Bacc("TRN2", target_bir_lowering=False)

    att1_d = nc.declare_dram_parameter("att1", [BPC, N, P, 2 * A], bf16,
                                       isOutput=False)
    obj_d = nc.declare_dram_parameter("obj", [BPC, O, 2, P], bf16, isOutput=False)
    # q pre-transposed on host to [2, P, BPC, N] (qc, qd_in, b, n)
    q_d = nc.declare_dram_parameter("q", [2, P, BPC, N], f32, isOutput=False)
    # tags pre-transposed on host to [P, BPC, N, AC] (a_in, b, n, c)
    tags_d = nc.declare_dram_parameter("tags", [P, BPC, N, AC], i32, isOutput=False)
    w1_d = nc.declare_dram_parameter("w1", [4, P, H], f32, isOutput=False)
    w2_d = nc.declare_dram_parameter("w2", [H, 1], f32, isOutput=False)
    b1_d = nc.declare_dram_parameter("b1", [H, 1], f32, isOutput=False)
    out_d = nc.declare_dram_parameter("out", [1, BPC * N * O], f32, isOutput=True)

    with TileContext(nc) as tc:
        with (
            tc.tile_pool(name="const", bufs=1) as constp,
            tc.tile_pool(name="att1b", bufs=8) as att1b_p,
            tc.tile_pool(name="joint", bufs=4) as joint_p,
            tc.tile_pool(name="small", bufs=6) as small_p,
            tc.tile_pool(name="perb", bufs=2) as perb_p,
            tc.tile_pool(name="pst", bufs=1, space="PSUM") as pst_p,
            tc.tile_pool(name="psj", bufs=3, space="PSUM") as psj_p,
            tc.tile_pool(name="psl", bufs=1, space="PSUM") as psl_p,
            tc.tile_pool(name="pss", bufs=1, space="PSUM") as pss_p,
        ):
            # ---- per-core constants / batched loads ----
            ident = constp.tile([P, P], bf16)
            make_identity(nc, ident)
            ones_f = constp.tile([P, P], f32)
            nc.vector.memset(ones_f, 1.0)

            def emit_setup():
                w1_f = constp.tile([P, 4, H], f32)
                nc.sync.dma_start(w1_f, w1_d[:].rearrange("c p h -> p c h"))
                w1_b = constp.tile([P, 4, H], bf16)
                nc.gpsimd.tensor_copy(w1_b, w1_f)

                w2_f = constp.tile([H, 1], f32)
                nc.sync.dma_start(w2_f, w2_d[:])
                w2_b = constp.tile([H, 1], bf16)
                nc.vector.tensor_copy(w2_b, w2_f)

                b1_sb = constp.tile([H, 1], f32)
                nc.sync.dma_start(b1_sb, b1_d[:])

                # all q for this core: [qd_in, qc, b, n]
                q_all_f = constp.tile([P, 2, BPC, N], f32)
                nc.sync.dma_start(q_all_f, q_d[:].rearrange("c p b n -> p c b n"))
                q_all_b = constp.tile([P, 2, BPC, N], bf16)
                nc.vector.tensor_copy(q_all_b, q_all_f)

                # tags -> additive mask (tag-1)*1e30 for all 32 pairs at once
                tags_sb = constp.tile([P, BPC, N, AC], i32)
                nc.sync.dma_start(tags_sb, tags_d[:])
                tags_ff = constp.tile([P, BPC, N, AC], f32)
                nc.vector.tensor_copy(tags_ff, tags_sb)
                negm = constp.tile([P, BPC, N, AC], f32)
                nc.vector.tensor_scalar(negm, tags_ff, -1.0, 1e30, OP.add, OP.mult)

                # all obj_reps for this core (host-bf16): [o, b, dc, d_in]
                obj_all_b = constp.tile([O, BPC, 2, P], bf16)
                nc.sync.dma_start(obj_all_b,
                                  obj_d[:].rearrange("b o c p -> o b c p"))
                return (w1_b, w2_b, b1_sb, obj_all_b, q_all_b, negm)

            # output staging: one row, DMA'd out once at the end
            out_acc = constp.tile([1, BPC * N * O], f32)

            def make_objW(b):
                """objW = obj[b] @ W1v  ([O, H] bf16)"""
                objT_b = perb_p.tile([P, 2, O], bf16, tag="objT")
                for c in range(2):
                    ps_t2 = pst_p.tile([P, P], bf16, tag="pst")
                    nc.tensor.transpose(ps_t2, obj_all_b[:, b, c, :], ident)
                    nc.vector.tensor_copy(objT_b[:, c, :], ps_t2)
                ps_w = pss_p.tile([P, P], f32, tag="bias")
                nc.tensor.matmul(ps_w[:, :H], objT_b[:, 0, :], w1_b[:, 0, :],
                                 start=True, stop=False)
                nc.tensor.matmul(ps_w[:, :H], objT_b[:, 1, :], w1_b[:, 1, :],
                                 start=False, stop=True)
                objW_b = perb_p.tile([O, H], bf16, tag="objW")
                nc.scalar.copy(objW_b, ps_w[:, :H])
                return objW_b

            def load_att1(b, n):
                att1_c = att1b_p.tile([P, 2, AC, O], bf16, tag="a1c")
                nc.sync.dma_start(att1_c, att1_d[b, n].rearrange("p (t a) -> p t a",
                                                                 t=2))
                return att1_c

            def stage_a(b, n, objW_b, att1_c):
                """bias -> jointT -> relu -> logits -> mask"""
                att1_b = att1_c[:, 0]   # natural: [a_in, (c, o)]
                att1T = att1_c[:, 1]    # transposed: [o, a] as [128, 8, 128]

                # bias column = q @ W1q + b1  ([H, 1])
                ps_bias = pss_p.tile([P, P], f32, tag="bias")
                nc.tensor.matmul(ps_bias[:, 0:1], w1_b[:, 2, :],
                                 q_all_b[:, 0, b, n][:, None],
                                 start=True, stop=False)
                nc.tensor.matmul(ps_bias[:, 0:1], w1_b[:, 3, :],
                                 q_all_b[:, 1, b, n][:, None],
                                 start=False, stop=True)
                bias_c = small_p.tile([H, 1], f32, tag="bias")
                nc.vector.tensor_scalar(bias_c, ps_bias[:, 0:1],
                                        b1_sb[:, 0:1], None, OP.add)

                jointT = joint_p.tile([H, AC, P], bf16, tag="joint")
                for half in range(2):
                    ps_j = psj_p.tile([H, 512], f32, tag="psj")
                    nc.tensor.matmul(ps_j, objW_b,
                                     att1T[:, half * 4:(half + 1) * 4, :],
                                     start=True, stop=True)
                    if half == 0:
                        nc.scalar.activation(
                            jointT[:, half * 4:(half + 1) * 4, :], ps_j,
                            AF.Relu, bias=bias_c[:, 0:1])
                    else:
                        nc.vector.tensor_scalar(
                            jointT[:, half * 4:(half + 1) * 4, :], ps_j,
                            bias_c[:, 0:1], 0.0, OP.add, OP.max)

                ps_l = psl_p.tile([P, AC], f32, tag="psl")
                for c in range(AC):
                    nc.tensor.matmul(ps_l[:, c:c + 1], jointT[:, c, :], w2_b,
                                     start=True, stop=True)

                masked = small_p.tile([P, AC], f32, tag="mask")
                nc.vector.tensor_tensor(masked, ps_l, negm[:, b, n, :], OP.add)
                return att1_b, masked

            def stage_b_exp(b, n, att1_b, masked):
                """exp of masked logits (ACT; emitted at iteration start)"""
                e_col = small_p.tile([P, AC], bf16, tag="ecol")
                s_col = small_p.tile([P, 1], f32, tag="scol")
                nc.scalar.activation(e_col, masked, AF.Exp, accum_out=s_col)
                return att1_b, e_col, s_col

            def stage_b_sum(att1_b, e_col, s_col):
                """S (scalar) -> recip; final einsum uses raw exp"""
                ps_S = pss_p.tile([1, 1], f32, tag="s")
                nc.tensor.matmul(ps_S, s_col, ones_f[:, 0:1],
                                 start=True, stop=True)
                recip = small_p.tile([1, 1], f32, tag="recip")
                nc.vector.reciprocal(recip, ps_S)
                return att1_b, e_col, recip

            def stage_c(b, n, att1_b, e_col, recip):
                """final einsum on raw exp -> scale by 1/S into out_acc"""
                ps_o = pss_p.tile([1, O], f32, tag="o")
                for c in range(AC):
                    nc.tensor.matmul(ps_o, e_col[:, c:c + 1], att1_b[:, c, :],
                                     start=(c == 0), stop=(c == AC - 1))
                idx = (b * N + n) * O
                nc.vector.tensor_scalar_mul(out_acc[0:1, idx:idx + O], ps_o,
                                            recip[0:1, 0:1])

            pairs = [(b, n) for b in range(BPC) for n in range(N)]
            objW_cache = {}
            pend_b = None
            pend_c = None
            LOOKAHEAD = 2
            loads = {}
            for i in range(LOOKAHEAD):
                loads[i] = load_att1(*pairs[i])
            w1_b, w2_b, b1_sb, obj_all_b, q_all_b, negm = emit_setup()
            for i, (b, n) in enumerate(pairs):
                if i + LOOKAHEAD < len(pairs):
                    loads[i + LOOKAHEAD] = load_att1(*pairs[i + LOOKAHEAD])
                if n == 0:
                    objW_cache[b] = make_objW(b)
                if pend_b is not None:
                    exp_state = stage_b_exp(*pend_b)
                else:
                    exp_state = None
                a_state = stage_a(b, n, objW_cache[b], loads.pop(i))
                if exp_state is not None:
                    b_state = pend_b[:2] + stage_b_sum(*exp_state)
                else:
                    b_state = None
                if pend_c is not None:
                    stage_c(*pend_c)
                pend_b = (b, n) + a_state
                pend_c = b_state
            b_state = pend_b[:2] + stage_b_sum(*stage_b_exp(*pend_b))
            stage_c(*pend_c)
            stage_c(*b_state)

            nc.sync.dma_start(out_d[:], out_acc)

    nc.compile()
    return nc


def _get_nc():
    global _NC
    if _NC is None:
        _NC = _build_nc()
    return _NC


def kernel(**inputs):
    q = np.asarray(inputs["q"], dtype=np.float32)
    att1 = np.asarray(inputs["att1"], dtype=np.float32)
    obj = np.asarray(inputs["obj_reps"], dtype=np.float32)
    tags = np.asarray(inputs["tags_attention"], dtype=np.int32)
    W1 = np.asarray(inputs["W1"], dtype=np.float32)
    b1 = np.asarray(inputs["b1"], dtype=np.float32)
    W2 = np.asarray(inputs["W2"], dtype=np.float32)
    t = float(np.asarray(inputs["t"]))
    # b2 dropped: constant shift is softmax-invariant.

    nc = _get_nc()
    from concourse.bass_utils import run_bass_kernel_spmd

    w1r = np.ascontiguousarray(W1.reshape(4, P, H))
    w2s = np.ascontiguousarray((W2 / t).reshape(H, 1).astype(np.float32))
    b1r = np.ascontiguousarray(b1.reshape(H, 1))

    import ml_dtypes
    att1_bf = att1.astype(ml_dtypes.bfloat16)
    att1_nat = att1_bf.reshape(B, N, AC, P, O).transpose(0, 1, 3, 2, 4) \
        .reshape(B, N, P, A)
    att1_t = att1_bf.transpose(0, 1, 3, 2)
    att1_c = np.concatenate([att1_nat, att1_t], axis=-1)

    in_maps = []
    for k in range(NCORES):
        bs = slice(k * BPC, (k + 1) * BPC)
        q_t = q[bs].reshape(BPC, N, 2, P).transpose(2, 3, 0, 1)
        tags_t = tags[bs].reshape(BPC, N, AC, P).transpose(3, 0, 1, 2)
        in_maps.append({
            "att1": np.ascontiguousarray(att1_c[bs]),
            "obj": np.ascontiguousarray(
                obj[bs].reshape(BPC, O, 2, P).astype(ml_dtypes.bfloat16)),
            "q": np.ascontiguousarray(q_t),
            "tags": np.ascontiguousarray(tags_t),
            "w1": w1r,
            "w2": w2s,
            "b1": b1r,
        })

    res = run_bass_kernel_spmd(nc, in_maps, core_ids=list(range(NCORES)),
                               trace=TRACE, **TRACE_KW)
    out = np.concatenate(
        [r["out"].reshape(BPC, N, O) for r in res.results], axis=0)
    if TRACE:
        print("HW exec time:", res.exec_time_ns, "ns",
              "(mean:", res.mean_exec_time_ns, ")")
        if res.instructions_and_trace:
            print("trace:", res.instructions_and_trace[1])
    return out



# revision 9
# speedup vs baseline: 1.4937x; 1.4937x over previous
"""Trainium2 Bass kernel for nn_Att_0_layer2 (sparse_attention).

Math (per (b, n) pair):
  v = att1 @ obj_reps                      # [A,O]@[O,D] -- never materialized:
  vq@W1 = v@W1v + q@W1q  ==>  att1 @ (obj_reps @ W1v) + (q @ W1q)
  jointT = relu(objW.T @ att1.T + bias)    # [H, A], objW = obj@W1v, bias = q@W1q + b1
  logits = jointT.T @ W2 (/t folded into W2 host-side; b2 dropped: softmax-invariant)
  att2 = softmax(logits masked by tags>0)
  out = att2 @ att1                        # [O]

Sparsity: tokens with tag==0 get -1e30 logits -> softmax weight 0 -> they
contribute NOTHING (neither to the softmax nor to the final einsum).  The
mask is host-visible, so att1 is compacted to the ~A/2 surviving rows
(padded to a multiple of 128; pad slots carry a -1e30 additive mask so they
exp to 0).  This cuts DMA (the bottleneck) and all per-pair compute ~0.6x.

Sharding: pure data parallel, B=64 split 8 ways (8 b's per core).
Device per pair i=(b,n):
  PE:  jointT = objW[b].T @ att1cT            (2 matmuls, free 512/128)
       logits: lhsT=jointT chunk, rhs=W2      (AC matmuls, free 1)
       final:  lhsT=att1c natural chunk, rhs=exp column -> ps_out[:, i]
               (AC matmuls, free 1 -- raw exp, normalization batched at end)
  ACT: relu chunks 0..AC-2 (bias col i), exp(masked)->e bf16
  DVE: relu chunk AC-1, mask add, S = reduce(e) -> s_all[:, i]
Batched at setup: bias_all = q@W1q + b1 for all 32 pairs; objW for all 8 b's
(from host-pretransposed obj, no PE transposes).  Batched at end: one
matmul s_all->per-pair sums, one reciprocal, transpose, one scaled copy,
one output DMA.
"""

import sys
import os
import numpy as np

sys.path.insert(0, "/opt/trn_rl_repo")

B, N, A, O, D, Q, H = 64, 4, 1024, 128, 256, 256, 128
NCORES = 8
BPC = B // NCORES   # batches per core
P = 128             # partitions
NP = BPC * N        # pairs per core (32)

TRACE = False
TRACE_KW = {}

_NC_CACHE = {}
_NC_LAST = None


def _build_nc(AC):
    """AC = number of 128-token chunks per pair after compaction."""
    import concourse.bacc as bacc
    import concourse.mybir as mybir
    from concourse.tile import TileContext
    from concourse.masks import make_identity

    f32 = mybir.dt.float32
    bf16 = mybir.dt.bfloat16
    AF = mybir.ActivationFunctionType
    OP = mybir.AluOpType
    AX = mybir.AxisListType

    ACU = AC * P
    ACT_C = max(1, AC - 1)  # relu chunks on ACT (from psj_a)
    DVE_C = AC - ACT_C      # relu chunks on DVE (0 or 1)

    nc = bacc.Bacc("TRN2", target_bir_lowering=False)

    # att1 per pair: [natural [a_in, AC, O] | transposed [o, ACU]] bf16
    att1_d = nc.declare_dram_parameter("att1", [BPC, N, P, 2 * ACU], bf16,
                                       isOutput=False)
    # obj pre-transposed on host: [dc, d_in, b, o]
    obj_d = nc.declare_dram_parameter("obj", [2, P, BPC, O], bf16,
                                      isOutput=False)
    # q pre-transposed on host to [qc, q_in, pair]
    q_d = nc.declare_dram_parameter("q", [2, P, NP], f32, isOutput=False)
    # additive mask (0 real / -1e30 pad), host layout [a_in, b, n, c]
    negm_d = nc.declare_dram_parameter("negm", [P, BPC, N, AC], f32,
                                       isOutput=False)
    w1_d = nc.declare_dram_parameter("w1", [4, P, H], f32, isOutput=False)
    w2_d = nc.declare_dram_parameter("w2", [H, 1], f32, isOutput=False)
    b1_d = nc.declare_dram_parameter("b1", [H, 1], f32, isOutput=False)
    out_d = nc.declare_dram_parameter("out", [NP, O], f32, isOutput=True)

    with TileContext(nc) as tc:
        with (
            tc.tile_pool(name="const", bufs=1) as constp,
            tc.tile_pool(name="att1b", bufs=6) as att1b_p,
            tc.tile_pool(name="joint", bufs=3) as joint_p,
            tc.tile_pool(name="small", bufs=4) as small_p,
            tc.tile_pool(name="psja", bufs=2, space="PSUM") as psja_p,
            tc.tile_pool(name="psjb", bufs=1, space="PSUM") as psjb_p,
            tc.tile_pool(name="psl", bufs=2, space="PSUM") as psl_p,
            tc.tile_pool(name="pso", bufs=1, space="PSUM") as pso_p,
        ):
            ones_f = constp.tile([P, 1], f32)
            nc.vector.memset(ones_f, 1.0)
            ident_f = constp.tile([P, P], f32)
            make_identity(nc, ident_f)

            # persistent accumulators
            ps_out = pso_p.tile([P, NP], f32)          # [o, pair]
            s_all = constp.tile([P, NP], f32)          # [a_in, pair] chunk-sums

            def load_att1(b, n):
                t = att1b_p.tile([P, 2, AC, O], bf16, tag="a1c")
                nc.sync.dma_start(t, att1_d[b, n].rearrange("p (t a) -> p t a",
                                                            t=2))
                return t

            def emit_setup():
                w1_f = constp.tile([P, 4, H], f32)
                nc.sync.dma_start(w1_f, w1_d[:].rearrange("c p h -> p c h"))
                w1_b = constp.tile([P, 4, H], bf16)
                nc.gpsimd.tensor_copy(w1_b, w1_f)

                q_all_f = constp.tile([P, 2, NP], f32)
                nc.sync.dma_start(q_all_f, q_d[:].rearrange("c p m -> p c m"))
                q_all_b = constp.tile([P, 2, NP], bf16)
                nc.vector.tensor_copy(q_all_b, q_all_f)

                objT = constp.tile([P, 2, BPC, O], bf16)
                nc.sync.dma_start(objT, obj_d[:].rearrange("c p b o -> p c b o"))

                negm = constp.tile([P, BPC, N, AC], f32)
                nc.sync.dma_start(negm, negm_d[:])

                w2_f = constp.tile([H, 1], f32)
                nc.sync.dma_start(w2_f, w2_d[:])
                w2_b = constp.tile([H, 1], bf16)
                nc.vector.tensor_copy(w2_b, w2_f)

                b1_sb = constp.tile([H, 1], f32)
                nc.sync.dma_start(b1_sb, b1_d[:])

                # objW[b] = obj[b] @ W1v for all b: [o, H] each, bf16
                # (psum slots shared with the per-pair "ja" ring)
                objW = constp.tile([P, BPC, H], bf16)
                for half in range(2):
                    ps_ow = psja_p.tile([P, 4 * H], f32, tag="ja")
                    for bb in range(4):
                        bg = half * 4 + bb
                        for c in range(2):
                            nc.tensor.matmul(
                                ps_ow[:, bb * H:(bb + 1) * H],
                                objT[:, c, bg, :], w1_b[:, c, :],
                                start=(c == 0), stop=(c == 1))
                    nc.scalar.activation(
                        objW[:, half * 4:(half + 1) * 4, :],
                        ps_ow[:].rearrange("p (b h) -> p b h", b=4),
                        AF.Copy)

                # bias_all[:, i] = W1q.T @ q_i + b1 for all pairs: [H, NP]
                ps_bias = psja_p.tile([H, NP], f32, tag="ja")
                for c in range(2):
                    nc.tensor.matmul(ps_bias, w1_b[:, 2 + c, :],
                                     q_all_b[:, c, :],
                                     start=(c == 0), stop=(c == 1))
                bias_all = constp.tile([H, NP], f32)
                nc.vector.tensor_scalar(bias_all, ps_bias, b1_sb[:, 0:1],
                                        None, OP.add)
                return w1_b, w2_b, objW, bias_all, negm

            def stage_a(i, b, n, objW, bias_all, negm, w2_b, att1_c):
                """joint -> relu -> logits -> mask.  Returns (att1_c, masked)."""
                # jointT chunks: [H, ACU] = objW[b].T @ att1T
                ps_ja = psja_p.tile([H, ACT_C * P], f32, tag="ja")
                nc.tensor.matmul(ps_ja, objW[:, b, :],
                                 att1_c[:, 1, 0:ACT_C, :],
                                 start=True, stop=True)
                if DVE_C:
                    ps_jb = psjb_p.tile([H, P], f32, tag="jb")
                    nc.tensor.matmul(ps_jb, objW[:, b, :],
                                     att1_c[:, 1, ACT_C, :],
                                     start=True, stop=True)

                jointT = joint_p.tile([H, AC, P], bf16, tag="joint")
                bcol = bias_all[:, i:i + 1]
                nc.scalar.activation(
                    jointT[:, 0:ACT_C, :],
                    ps_ja[:].rearrange("p (c a) -> p c a", c=ACT_C),
                    AF.Relu, bias=bcol)
                if DVE_C:
                    nc.vector.tensor_scalar(jointT[:, ACT_C, :], ps_jb,
                                            bcol, 0.0, OP.add, OP.max)

                ps_l = psl_p.tile([P, AC], f32, tag="psl")
                for c in range(AC):
                    nc.tensor.matmul(ps_l[:, c:c + 1], jointT[:, c, :], w2_b,
                                     start=True, stop=True)

                # mask in place: pad/tag-0 slots get -1e30
                nc.vector.tensor_tensor(ps_l, ps_l, negm[:, b, n, :], OP.add)
                return att1_c, ps_l

            def stage_b(i, att1_c, masked):
                """exp (ACT) + chunk-sum reduce (DVE)."""
                e_col = small_p.tile([P, AC], bf16, tag="ecol")
                nc.scalar.activation(e_col, masked, AF.Exp)
                nc.vector.tensor_reduce(s_all[:, i:i + 1], e_col, AX.X, OP.add)
                return att1_c, e_col

            def stage_c(i, att1_c, e_col):
                """raw-exp einsum into ps_out[:, i] (ap_size=1 matmuls)."""
                for c in range(AC):
                    nc.tensor.matmul(ps_out[:, i:i + 1], att1_c[:, 0, c, :],
                                     e_col[:, c:c + 1],
                                     start=(c == 0), stop=(c == AC - 1))

            pairs = [(b, n) for b in range(BPC) for n in range(N)]
            LOOKAHEAD = 6
            # setup DMAs first: objW/bias inputs must land before pair 0
            w1_b, w2_b, objW, bias_all, negm = emit_setup()
            loads = {}
            for j in range(LOOKAHEAD):
                loads[j] = load_att1(*pairs[j])

            pend_b = None   # (i, att1_c, masked) awaiting exp
            pend_c = None   # (i, att1_c, e_col) awaiting final einsum
            for i, (b, n) in enumerate(pairs):
                if i + LOOKAHEAD < len(pairs):
                    loads[i + LOOKAHEAD] = load_att1(*pairs[i + LOOKAHEAD])
                a_state = stage_a(i, b, n, objW, bias_all, negm, w2_b,
                                  loads.pop(i))
                if pend_c is not None:
                    stage_c(*pend_c)
                if pend_b is not None:
                    pend_c = (pend_b[0],) + stage_b(*pend_b)
                else:
                    pend_c = None
                pend_b = (i,) + a_state
            pend_c2 = (pend_b[0],) + stage_b(*pend_b)
            if pend_c is not None:
                stage_c(*pend_c)
            stage_c(*pend_c2)

            # ---- batched softmax normalization + output ----
            # (psum slots shared with the per-pair "psl" ring)
            ps_ssum = psl_p.tile([NP, 1], f32, tag="psl")
            nc.tensor.matmul(ps_ssum, s_all, ones_f, start=True, stop=True)
            recip = constp.tile([NP, 1], f32)
            nc.vector.reciprocal(recip, ps_ssum)

            out_cols = constp.tile([P, NP], f32)
            nc.vector.tensor_copy(out_cols, ps_out)
            ps_outT = psl_p.tile([NP, P], f32, tag="psl")
            nc.tensor.transpose(ps_outT, out_cols, ident_f)
            out_sb = constp.tile([NP, O], f32)
            nc.scalar.activation(out_sb, ps_outT, AF.Copy, scale=recip)
            nc.sync.dma_start(out_d[:], out_sb)

    nc.compile()
    return nc


def _get_nc(AC=None):
    global _NC_LAST
    if AC is None:
        if _NC_LAST is not None:
            return _NC_LAST
        AC = 5
    if AC not in _NC_CACHE:
        _NC_CACHE[AC] = _build_nc(AC)
    _NC_LAST = _NC_CACHE[AC]
    return _NC_LAST


def kernel(**inputs):
    q = np.asarray(inputs["q"], dtype=np.float32)
    att1 = np.asarray(inputs["att1"], dtype=np.float32)
    obj = np.asarray(inputs["obj_reps"], dtype=np.float32)
    tags = np.asarray(inputs["tags_attention"], dtype=np.int32)
    W1 = np.asarray(inputs["W1"], dtype=np.float32)
    b1 = np.asarray(inputs["b1"], dtype=np.float32)
    W2 = np.asarray(inputs["W2"], dtype=np.float32)
    t = float(np.asarray(inputs["t"]))
    # b2 dropped: constant shift is softmax-invariant.

    import ml_dtypes

    # ---- sparsity compaction: keep only tag==1 rows of att1 ----
    cnt = tags.sum(axis=-1)                      # [B, N]
    AC = max(1, int(-(-int(cnt.max()) // P)))    # chunks of 128
    ACU = AC * P
    order = np.argsort(1 - tags, axis=-1, kind="stable")[..., :ACU]  # [B,N,ACU]
    att1_comp = np.take_along_axis(att1, order[..., None], axis=2)   # [B,N,ACU,O]
    valid = np.take_along_axis(tags, order, axis=2)                  # [B,N,ACU]
    negm_full = (valid.astype(np.float32) - 1.0) * 1e30              # 0 / -1e30

    att1_bf = att1_comp.astype(ml_dtypes.bfloat16)
    nat = att1_bf.reshape(B, N, AC, P, O).transpose(0, 1, 3, 2, 4) \
        .reshape(B, N, P, ACU)
    trans = att1_bf.transpose(0, 1, 3, 2)                            # [B,N,O,ACU]
    att1_c = np.concatenate([nat, trans], axis=-1)                   # [B,N,128,2ACU]

    nc = _get_nc(AC)
    from concourse.bass_utils import run_bass_kernel_spmd

    w1r = np.ascontiguousarray(W1.reshape(4, P, H))
    w2s = np.ascontiguousarray((W2 / t).reshape(H, 1).astype(np.float32))
    b1r = np.ascontiguousarray(b1.reshape(H, 1))

    in_maps = []
    for k in range(NCORES):
        bs = slice(k * BPC, (k + 1) * BPC)
        q_t = q[bs].reshape(BPC * N, 2, P).transpose(1, 2, 0)        # [2,P,NP]
        objT_t = obj[bs].transpose(2, 0, 1).reshape(2, P, BPC, O) \
            .astype(ml_dtypes.bfloat16)                              # [2,P,BPC,O]
        negm_t = negm_full[bs].reshape(BPC, N, AC, P).transpose(3, 0, 1, 2)
        in_maps.append({
            "att1": np.ascontiguousarray(att1_c[bs]),
            "obj": np.ascontiguousarray(objT_t),
            "q": np.ascontiguousarray(q_t),
            "negm": np.ascontiguousarray(negm_t),
            "w1": w1r,
            "w2": w2s,
            "b1": b1r,
        })

    res = run_bass_kernel_spmd(nc, in_maps, core_ids=list(range(NCORES)),
                               trace=TRACE, **TRACE_KW)
    out = np.concatenate(
        [r["out"].reshape(BPC, N, O) for r in res.results], axis=0)
    if TRACE:
        print("HW exec time:", res.exec_time_ns, "ns",
              "(mean:", res.mean_exec_time_ns, ")")
        if res.instructions_and_trace:
            print("trace:", res.instructions_and_trace[1])
    return out


# revision 13
# speedup vs baseline: 1.5282x; 1.0231x over previous
"""Trainium2 Bass kernel for nn_Att_0_layer2 (sparse_attention).

Math (per (b, n) pair):
  v = att1 @ obj_reps                      # [A,O]@[O,D] -- never materialized:
  vq@W1 = v@W1v + q@W1q  ==>  att1 @ (obj_reps @ W1v) + (q @ W1q)
  jointT = relu(objW.T @ att1.T + bias)    # [H, A], objW = obj@W1v, bias = q@W1q + b1
  logits = jointT.T @ W2 (/t folded into W2 host-side; b2 dropped: softmax-invariant)
  att2 = softmax(logits masked by tags>0)
  out = att2 @ att1                        # [O]

Sparsity: tokens with tag==0 get -1e30 logits -> softmax weight 0 -> they
contribute NOTHING downstream.  The mask is host-visible, so att1 is
compacted to the ~A/2 surviving rows (padded to a multiple of 128; pad
slots carry a -1e30 additive mask so they exp to 0).  Cuts DMA (the
bottleneck) and all per-pair compute ~0.6x.

Sharding: pure data parallel, B=64 split 8 ways (8 b's per core).
Device, per pair group (2 pairs per att1 DMA):
  PE:  jointT chunks = objW[b].T @ att1T   (2 matmuls)
       logits: lhsT=jointT chunk, rhs=W2   (AC matmuls, free-size 1)
       final:  lhsT=att1 natural chunk, rhs=exp col -> ps_out[:, i]
               (AC matmuls, free-size 1; raw exp, normalized on host)
  ACT: relu chunks 0..AC-3 (bias col i); exp of both pairs' masked logits
  DVE: relu last 2 chunks, mask add, S-reduce -> outbuf[:, NP+i]
Batched at setup: bias_all (all 32 pairs), objW (all 8 b's, from
host-pretransposed obj).  Output: one [128, 2*NP] DMA of raw out columns +
exp-sums; host does out[i,:] = cols[:,i] / sum(s[:,i]).
"""

import sys
import os
import numpy as np

sys.path.insert(0, "/opt/trn_rl_repo")

B, N, A, O, D, Q, H = 64, 4, 1024, 128, 256, 256, 128
NCORES = 8
BPC = B // NCORES   # batches per core
P = 128             # partitions
NP = BPC * N        # pairs per core (32)
NG = NP // 2        # pair groups (2 pairs per group)

TRACE = False
TRACE_KW = {}

_NC_CACHE = {}
_NC_LAST = None


def _build_nc(AC):
    """AC = number of 128-token chunks per pair after compaction."""
    import concourse.bacc as bacc
    import concourse.mybir as mybir
    from concourse.tile import TileContext

    f32 = mybir.dt.float32
    bf16 = mybir.dt.bfloat16
    AF = mybir.ActivationFunctionType
    OP = mybir.AluOpType
    AX = mybir.AxisListType

    ACU = AC * P
    ACT_C = max(1, AC - 2)  # relu chunks on ACT (from ps_ja)
    DVE_C = AC - ACT_C      # relu chunks on DVE (from ps_jb)

    nc = bacc.Bacc("TRN2", target_bir_lowering=False)

    # att1 per pair: [natural [a_in, AC, O] | transposed [o, ACU]] bf16
    att1_d = nc.declare_dram_parameter("att1", [BPC, N, P, 2 * ACU], bf16,
                                       isOutput=False)
    # obj pre-transposed on host: [dc, d_in, b, o]
    obj_d = nc.declare_dram_parameter("obj", [2, P, BPC, O], bf16,
                                      isOutput=False)
    # q pre-transposed on host to [qc, q_in, pair]
    q_d = nc.declare_dram_parameter("q", [2, P, NP], bf16, isOutput=False)
    # additive mask (0 real / -1e30 pad), host layout [a_in, b, n, c]
    negm_d = nc.declare_dram_parameter("negm", [P, BPC, N, AC], f32,
                                       isOutput=False)
    w1_d = nc.declare_dram_parameter("w1", [4, P, H], bf16, isOutput=False)
    w2_d = nc.declare_dram_parameter("w2", [H, 1], bf16, isOutput=False)
    b1_d = nc.declare_dram_parameter("b1", [H, 1], f32, isOutput=False)
    # raw output columns [o, pair] and exp-sums [a_in, pair]
    outs_d = nc.declare_dram_parameter("outs", [P, 2 * NP], f32, isOutput=True)

    with TileContext(nc) as tc:
        with (
            tc.tile_pool(name="const", bufs=1) as constp,
            tc.tile_pool(name="att1b", bufs=6) as att1b_p,
            tc.tile_pool(name="joint", bufs=3) as joint_p,
            tc.tile_pool(name="small", bufs=3) as small_p,
            tc.tile_pool(name="psja", bufs=3, space="PSUM") as psja_p,
            tc.tile_pool(name="psjb", bufs=2, space="PSUM") as psjb_p,
            tc.tile_pool(name="psl", bufs=2, space="PSUM") as psl_p,
            tc.tile_pool(name="pso", bufs=1, space="PSUM") as pso_p,
        ):
            # persistent accumulators / output staging
            ps_out = pso_p.tile([P, NP], f32)          # [o, pair]
            outbuf = constp.tile([P, 2 * NP], f32)     # [:, :NP]=cols, [NP:]=s

            def load_att1(g):
                t = att1b_p.tile([P, 2, 2, AC, O], bf16, tag="a1c")
                b, n = divmod(2 * g, N)
                nc.sync.dma_start(
                    t, att1_d[b, n:n + 2].rearrange(
                        "n p (t c a) -> p n t c a", t=2, c=AC))
                return t

            def emit_setup():
                w1_b = constp.tile([P, 4, H], bf16)
                nc.sync.dma_start(w1_b, w1_d[:].rearrange("c p h -> p c h"))

                objT = constp.tile([P, 2, BPC, O], bf16)
                nc.sync.dma_start(objT, obj_d[:].rearrange("c p b o -> p c b o"))

                q_all_b = constp.tile([P, 2, NP], bf16)
                nc.sync.dma_start(q_all_b, q_d[:].rearrange("c p m -> p c m"))

                negm = constp.tile([P, BPC, N, AC], f32)
                nc.sync.dma_start(negm, negm_d[:])

                w2_b = constp.tile([H, 1], bf16)
                nc.sync.dma_start(w2_b, w2_d[:])
                b1_sb = constp.tile([H, 1], f32)
                nc.sync.dma_start(b1_sb, b1_d[:])

                # objW[b] = obj[b] @ W1v for all b: [o, H] each, bf16
                # (psum slots shared with the per-pair "ja" ring)
                objW = constp.tile([P, BPC, H], bf16)
                for half in range(2):
                    ps_ow = psja_p.tile([P, 4 * H], f32, tag="ja")
                    for bb in range(4):
                        bg = half * 4 + bb
                        for c in range(2):
                            nc.tensor.matmul(
                                ps_ow[:, bb * H:(bb + 1) * H],
                                objT[:, c, bg, :], w1_b[:, c, :],
                                start=(c == 0), stop=(c == 1))
                    nc.scalar.activation(
                        objW[:, half * 4:(half + 1) * 4, :],
                        ps_ow[:].rearrange("p (b h) -> p b h", b=4),
                        AF.Copy)

                # bias_all[:, i] = W1q.T @ q_i + b1 for all pairs: [H, NP]
                ps_bias = psja_p.tile([H, NP], f32, tag="ja")
                for c in range(2):
                    nc.tensor.matmul(ps_bias, w1_b[:, 2 + c, :],
                                     q_all_b[:, c, :],
                                     start=(c == 0), stop=(c == 1))
                bias_all = constp.tile([H, NP], f32)
                nc.vector.tensor_scalar(bias_all, ps_bias, b1_sb[:, 0:1],
                                        None, OP.add)
                return w1_b, w2_b, objW, bias_all, negm

            def joint_mm(i, b, objW, att1_c, j):
                ps_ja = psja_p.tile([H, ACT_C * P], f32, tag="ja")
                nc.tensor.matmul(ps_ja, objW[:, b, :],
                                 att1_c[:, j, 1, 0:ACT_C, :],
                                 start=True, stop=True)
                ps_jb = psjb_p.tile([H, DVE_C * P], f32, tag="jb")
                nc.tensor.matmul(ps_jb, objW[:, b, :],
                                 att1_c[:, j, 1, ACT_C:AC, :],
                                 start=True, stop=True)
                return ps_ja, ps_jb

            def relu_act(i, bias_all, ps_ja, jointT):
                nc.scalar.activation(
                    jointT[:, 0:ACT_C, :],
                    ps_ja[:].rearrange("p (c a) -> p c a", c=ACT_C),
                    AF.Relu, bias=bias_all[:, i:i + 1])

            def relu_dve(i, bias_all, ps_jb, jointT):
                nc.vector.tensor_scalar(
                    jointT[:, ACT_C:AC, :],
                    ps_jb[:].rearrange("p (c a) -> p c a", c=DVE_C),
                    bias_all[:, i:i + 1], 0.0, OP.add, OP.max)

            def logits_mm(jointT, w2_b):
                ps_l = psl_p.tile([P, AC], f32, tag="psl")
                for c in range(AC):
                    nc.tensor.matmul(ps_l[:, c:c + 1], jointT[:, c, :], w2_b,
                                     start=True, stop=True)
                return ps_l

            def mask_add(b, n, negm, ps_l, masked2, j):
                nc.vector.tensor_tensor(masked2[:, j, :], ps_l,
                                        negm[:, b, n, :], OP.add)

            def exp_reduce(g, masked2):
                e2 = small_p.tile([P, 2, AC], bf16, tag="e2")
                nc.scalar.activation(e2, masked2, AF.Exp)
                i0 = 2 * g
                nc.vector.tensor_reduce(outbuf[:, NP + i0:NP + i0 + 2], e2,
                                        AX.X, OP.add)
                return e2

            def final_mm(g, att1_c, e2):
                for j in range(2):
                    i = 2 * g + j
                    for c in range(AC):
                        nc.tensor.matmul(ps_out[:, i:i + 1],
                                         att1_c[:, j, 0, c, :],
                                         e2[:, j, c:c + 1],
                                         start=(c == 0), stop=(c == AC - 1))

            # ---- emission ----
            LOOKAHEAD = 3                      # groups prefetched ahead
            w1_b, w2_b, objW, bias_all, negm = emit_setup()
            loads = {g: load_att1(g) for g in range(LOOKAHEAD)}

            pend_ab = {}    # g -> (att1_c, masked2)
            pend_c = {}     # g -> (att1_c, e2)
            for g in range(NG):
                if g + LOOKAHEAD < NG:
                    loads[g + LOOKAHEAD] = load_att1(g + LOOKAHEAD)
                att1_c = loads.pop(g)
                i0 = 2 * g
                b0, n0 = divmod(i0, N)
                b1_, n1 = divmod(i0 + 1, N)
                jointT0 = joint_p.tile([H, AC, P], bf16, tag="joint")
                jointT1 = joint_p.tile([H, AC, P], bf16, tag="joint")
                masked2 = small_p.tile([P, 2, AC], f32, tag="mask")

                ja0, jb0 = joint_mm(i0, b0, objW, att1_c, 0)
                relu_act(i0, bias_all, ja0, jointT0)
                relu_dve(i0, bias_all, jb0, jointT0)
                if g >= 2:
                    final_mm(g - 2, *pend_c.pop(g - 2))
                ja1, jb1 = joint_mm(i0 + 1, b1_, objW, att1_c, 1)
                relu_act(i0 + 1, bias_all, ja1, jointT1)
                relu_dve(i0 + 1, bias_all, jb1, jointT1)
                l0 = logits_mm(jointT0, w2_b)
                mask_add(b0, n0, negm, l0, masked2, 0)
                l1 = logits_mm(jointT1, w2_b)
                mask_add(b1_, n1, negm, l1, masked2, 1)
                if g >= 1:
                    gp = g - 1
                    pend_c[gp] = (pend_ab[gp][0], exp_reduce(gp,
                                                            pend_ab.pop(gp)[1]))
                pend_ab[g] = (att1_c, masked2)

            g = NG - 1
            pend_c[g] = (pend_ab[g][0], exp_reduce(g, pend_ab.pop(g)[1]))
            final_mm(g - 1, *pend_c.pop(g - 1))
            final_mm(g, *pend_c.pop(g))

            nc.vector.tensor_copy(outbuf[:, 0:NP], ps_out)
            nc.sync.dma_start(outs_d[:], outbuf)

    nc.compile()
    return nc


def _get_nc(AC=None):
    global _NC_LAST
    if AC is None:
        if _NC_LAST is not None:
            return _NC_LAST
        AC = 5
    if AC not in _NC_CACHE:
        _NC_CACHE[AC] = _build_nc(AC)
    _NC_LAST = _NC_CACHE[AC]
    return _NC_LAST


def kernel(**inputs):
    q = np.asarray(inputs["q"], dtype=np.float32)
    att1 = np.asarray(inputs["att1"], dtype=np.float32)
    obj = np.asarray(inputs["obj_reps"], dtype=np.float32)
    tags = np.asarray(inputs["tags_attention"], dtype=np.int32)
    W1 = np.asarray(inputs["W1"], dtype=np.float32)
    b1 = np.asarray(inputs["b1"], dtype=np.float32)
    W2 = np.asarray(inputs["W2"], dtype=np.float32)
    t = float(np.asarray(inputs["t"]))
    # b2 dropped: constant shift is softmax-invariant.

    import ml_dtypes

    # ---- sparsity compaction: keep only tag==1 rows of att1 ----
    cnt = tags.sum(axis=-1)                      # [B, N]
    AC = max(2, int(-(-int(cnt.max()) // P)))    # chunks of 128
    ACU = AC * P
    order = np.argsort(1 - tags, axis=-1, kind="stable")[..., :ACU]  # [B,N,ACU]
    att1_comp = np.take_along_axis(att1, order[..., None], axis=2)   # [B,N,ACU,O]
    valid = np.take_along_axis(tags, order, axis=2)                  # [B,N,ACU]
    negm_full = (valid.astype(np.float32) - 1.0) * 1e30              # 0 / -1e30

    att1_bf = att1_comp.astype(ml_dtypes.bfloat16)
    nat = att1_bf.reshape(B, N, AC, P, O).transpose(0, 1, 3, 2, 4) \
        .reshape(B, N, P, ACU)
    trans = att1_bf.transpose(0, 1, 3, 2)                            # [B,N,O,ACU]
    att1_c = np.concatenate([nat, trans], axis=-1)                   # [B,N,128,2ACU]

    nc = _get_nc(AC)
    from concourse.bass_utils import run_bass_kernel_spmd

    w1r = np.ascontiguousarray(W1.reshape(4, P, H)).astype(ml_dtypes.bfloat16)
    w2s = np.ascontiguousarray((W2 / t).reshape(H, 1)).astype(ml_dtypes.bfloat16)
    b1r = np.ascontiguousarray(b1.reshape(H, 1))

    in_maps = []
    for k in range(NCORES):
        bs = slice(k * BPC, (k + 1) * BPC)
        q_t = q[bs].reshape(BPC * N, 2, P).transpose(1, 2, 0) \
            .astype(ml_dtypes.bfloat16)                              # [2,P,NP]
        objT_t = obj[bs].transpose(2, 0, 1).reshape(2, P, BPC, O) \
            .astype(ml_dtypes.bfloat16)                              # [2,P,BPC,O]
        negm_t = negm_full[bs].reshape(BPC, N, AC, P).transpose(3, 0, 1, 2)
        in_maps.append({
            "att1": np.ascontiguousarray(att1_c[bs]),
            "obj": np.ascontiguousarray(objT_t),
            "q": np.ascontiguousarray(q_t),
            "negm": np.ascontiguousarray(negm_t),
            "w1": w1r,
            "w2": w2s,
            "b1": b1r,
        })

    res = run_bass_kernel_spmd(nc, in_maps, core_ids=list(range(NCORES)),
                               trace=TRACE, **TRACE_KW)
    outs = []
    for r in res.results:
        raw = r["outs"]                          # [P, 2*NP] f32
        cols = raw[:, :NP]                       # [o, pair]
        s = raw[:, NP:].sum(axis=0)              # [pair]
        outs.append((cols / s[None, :]).T.reshape(BPC, N, O))
    out = np.concatenate(outs, axis=0)
    if TRACE:
        print("HW exec time:", res.exec_time_ns, "ns",
              "(mean:", res.mean_exec_time_ns, ")")
        if res.instructions_and_trace:
            print("trace:", res.instructions_and_trace[1])
    return out


# revision 17
# speedup vs baseline: 1.5290x; 1.0005x over previous
"""Trainium2 Bass kernel for nn_Att_0_layer2 (sparse_attention).

Math (per (b, n) pair):
  v = att1 @ obj_reps                      # [A,O]@[O,D] -- never materialized:
  vq@W1 = v@W1v + q@W1q  ==>  att1 @ (obj_reps @ W1v) + (q @ W1q)
  jointT = relu(objW.T @ att1.T + bias)    # [H, A], objW = obj@W1v, bias = q@W1q + b1
  logits = jointT.T @ W2 (/t folded into W2 host-side; b2 dropped: softmax-invariant)
  att2 = softmax(logits masked by tags>0)
  out = att2 @ att1                        # [O]

Sparsity: tokens with tag==0 get -1e30 logits -> softmax weight 0 -> they
contribute NOTHING downstream.  The mask is host-visible, so att1 is
compacted to the ~A/2 surviving rows (padded to a multiple of 128; pad
slots carry a -1e30 additive mask so they exp to 0).  Cuts DMA (the
bottleneck) and all per-pair compute ~0.6x.

Sharding: pure data parallel, B=64 split 8 ways (8 b's per core).
Device, per pair group (2 pairs per att1 DMA):
  PE:  jointT chunks = objW[b].T @ att1T   (2 matmuls)
       logits: lhsT=jointT chunk, rhs=W2   (AC matmuls, free-size 1)
       final:  lhsT=att1 natural chunk, rhs=exp col -> ps_out[:, i]
               (AC matmuls, free-size 1; raw exp, normalized on host)
  ACT: relu chunks 0..AC-3 (bias col i); exp of both pairs' masked logits
  DVE: relu last 2 chunks, mask add, S-reduce -> outbuf[:, NP+i]
Batched at setup: bias_all (all 32 pairs), objW (all 8 b's, from
host-pretransposed obj).  Output: one [128, 2*NP] DMA of raw out columns +
exp-sums; host does out[i,:] = cols[:,i] / sum(s[:,i]).
"""

import sys
import os
import numpy as np

sys.path.insert(0, "/opt/trn_rl_repo")

B, N, A, O, D, Q, H = 64, 4, 1024, 128, 256, 256, 128
NCORES = 8
BPC = B // NCORES   # batches per core
P = 128             # partitions
NP = BPC * N        # pairs per core (32)
NG = NP // 2        # pair groups (2 pairs per group)

TRACE = False
TRACE_KW = {}

_NC_CACHE = {}
_NC_LAST = None


def _build_nc(AC):
    """AC = number of 128-token chunks per pair after compaction."""
    import concourse.bacc as bacc
    import concourse.mybir as mybir
    from concourse.tile import TileContext

    f32 = mybir.dt.float32
    bf16 = mybir.dt.bfloat16
    AF = mybir.ActivationFunctionType
    OP = mybir.AluOpType
    AX = mybir.AxisListType

    ACU = AC * P
    ACT_C = max(1, AC - 2)  # relu chunks on ACT (from ps_ja)
    DVE_C = AC - ACT_C      # relu chunks on DVE (from ps_jb)

    nc = bacc.Bacc("TRN2", target_bir_lowering=False)

    # att1 per pair: [natural [a_in, AC, O] | transposed [o, ACU]] bf16
    att1_d = nc.declare_dram_parameter("att1", [BPC, N, P, 2 * ACU], bf16,
                                       isOutput=False)
    # obj pre-transposed on host: [dc, d_in, b, o]
    obj_d = nc.declare_dram_parameter("obj", [2, P, BPC, O], bf16,
                                      isOutput=False)
    # q pre-transposed on host to [qc, q_in, pair]
    q_d = nc.declare_dram_parameter("q", [2, P, NP], bf16, isOutput=False)
    # additive mask (0 real / -1e30 pad), host layout [a_in, b, n, c]
    negm_d = nc.declare_dram_parameter("negm", [P, BPC, N, AC], f32,
                                       isOutput=False)
    w1_d = nc.declare_dram_parameter("w1", [P, 4, H], bf16, isOutput=False)
    w2_d = nc.declare_dram_parameter("w2", [H, 1], bf16, isOutput=False)
    b1_d = nc.declare_dram_parameter("b1", [H, 1], f32, isOutput=False)
    # raw output columns [o, pair] and exp-sums [a_in, pair]
    outs_d = nc.declare_dram_parameter("outs", [P, 2 * NP], f32, isOutput=True)

    with TileContext(nc) as tc:
        with (
            tc.tile_pool(name="const", bufs=1) as constp,
            tc.tile_pool(name="att1b", bufs=6) as att1b_p,
            tc.tile_pool(name="joint", bufs=3) as joint_p,
            tc.tile_pool(name="small", bufs=3) as small_p,
            tc.tile_pool(name="psja", bufs=3, space="PSUM") as psja_p,
            tc.tile_pool(name="psjb", bufs=2, space="PSUM") as psjb_p,
            tc.tile_pool(name="psl", bufs=2, space="PSUM") as psl_p,
            tc.tile_pool(name="pso", bufs=1, space="PSUM") as pso_p,
        ):
            # persistent accumulators / output staging
            ps_out = pso_p.tile([P, NP], f32)          # [o, pair]
            outbuf = constp.tile([P, 2 * NP], f32)     # [:, :NP]=cols, [NP:]=s

            def load_att1(g):
                t = att1b_p.tile([P, 2, 2, AC, O], bf16, tag="a1c")
                b, n = divmod(2 * g, N)
                nc.sync.dma_start(
                    t, att1_d[b, n:n + 2].rearrange(
                        "n p (t c a) -> p n t c a", t=2, c=AC))
                return t

            def emit_setup():
                w1_b = constp.tile([P, 4, H], bf16)
                nc.sync.dma_start(w1_b, w1_d[:])

                objT = constp.tile([P, 2, BPC, O], bf16)
                nc.sync.dma_start(objT, obj_d[:].rearrange("c p b o -> p c b o"))

                q_all_b = constp.tile([P, 2, NP], bf16)
                nc.sync.dma_start(q_all_b, q_d[:].rearrange("c p m -> p c m"))

                negm = constp.tile([P, BPC, N, AC], f32)
                nc.sync.dma_start(negm, negm_d[:])

                w2_b = constp.tile([H, 1], bf16)
                nc.sync.dma_start(w2_b, w2_d[:])
                b1_sb = constp.tile([H, 1], f32)
                nc.sync.dma_start(b1_sb, b1_d[:])

                # objW[b] = obj[b] @ W1v for all b: [o, H] each, bf16
                # (psum slots shared with the per-pair "ja" ring)
                objW = constp.tile([P, BPC, H], bf16)
                for half in range(2):
                    ps_ow = psja_p.tile([P, 4 * H], f32, tag="ja")
                    for bb in range(4):
                        bg = half * 4 + bb
                        for c in range(2):
                            nc.tensor.matmul(
                                ps_ow[:, bb * H:(bb + 1) * H],
                                objT[:, c, bg, :], w1_b[:, c, :],
                                start=(c == 0), stop=(c == 1))
                    nc.scalar.activation(
                        objW[:, half * 4:(half + 1) * 4, :],
                        ps_ow[:].rearrange("p (b h) -> p b h", b=4),
                        AF.Copy)

                # bias_all[:, i] = W1q.T @ q_i + b1 for all pairs: [H, NP]
                ps_bias = psja_p.tile([H, NP], f32, tag="ja")
                for c in range(2):
                    nc.tensor.matmul(ps_bias, w1_b[:, 2 + c, :],
                                     q_all_b[:, c, :],
                                     start=(c == 0), stop=(c == 1))
                bias_all = constp.tile([H, NP], f32)
                nc.vector.tensor_scalar(bias_all, ps_bias, b1_sb[:, 0:1],
                                        None, OP.add)
                return w1_b, w2_b, objW, bias_all, negm

            def joint_mm(i, b, objW, att1_c, j):
                ps_ja = psja_p.tile([H, ACT_C * P], f32, tag="ja")
                nc.tensor.matmul(ps_ja, objW[:, b, :],
                                 att1_c[:, j, 1, 0:ACT_C, :],
                                 start=True, stop=True)
                ps_jb = psjb_p.tile([H, DVE_C * P], f32, tag="jb")
                nc.tensor.matmul(ps_jb, objW[:, b, :],
                                 att1_c[:, j, 1, ACT_C:AC, :],
                                 start=True, stop=True)
                return ps_ja, ps_jb

            def relu_act(i, bias_all, ps_ja, jointT):
                nc.scalar.activation(
                    jointT[:, 0:ACT_C, :],
                    ps_ja[:].rearrange("p (c a) -> p c a", c=ACT_C),
                    AF.Relu, bias=bias_all[:, i:i + 1])

            def relu_dve(i, bias_all, ps_jb, jointT):
                nc.vector.tensor_scalar(
                    jointT[:, ACT_C:AC, :],
                    ps_jb[:].rearrange("p (c a) -> p c a", c=DVE_C),
                    bias_all[:, i:i + 1], 0.0, OP.add, OP.max)

            def logits_mm(jointT, w2_b):
                ps_l = psl_p.tile([P, AC], f32, tag="psl")
                for c in range(AC):
                    nc.tensor.matmul(ps_l[:, c:c + 1], jointT[:, c, :], w2_b,
                                     start=True, stop=True)
                return ps_l

            def mask_add(b, n, negm, ps_l, masked2, j):
                nc.vector.tensor_tensor(masked2[:, j, :], ps_l,
                                        negm[:, b, n, :], OP.add)

            def exp_reduce(g, masked2):
                e2 = small_p.tile([P, 2, AC], bf16, tag="e2")
                nc.scalar.activation(e2, masked2, AF.Exp)
                i0 = 2 * g
                nc.vector.tensor_reduce(outbuf[:, NP + i0:NP + i0 + 2], e2,
                                        AX.X, OP.add)
                return e2

            def final_mm(g, att1_c, e2):
                for j in range(2):
                    i = 2 * g + j
                    for c in range(AC):
                        nc.tensor.matmul(ps_out[:, i:i + 1],
                                         att1_c[:, j, 0, c, :],
                                         e2[:, j, c:c + 1],
                                         start=(c == 0), stop=(c == AC - 1))

            # ---- emission ----
            LOOKAHEAD = 3                      # groups prefetched ahead
            w1_b, w2_b, objW, bias_all, negm = emit_setup()
            loads = {g: load_att1(g) for g in range(LOOKAHEAD)}

            pend_ab = {}    # g -> (att1_c, masked2)
            for g in range(NG):
                if g + LOOKAHEAD < NG:
                    loads[g + LOOKAHEAD] = load_att1(g + LOOKAHEAD)
                att1_c = loads.pop(g)
                i0 = 2 * g
                b0, n0 = divmod(i0, N)
                b1_, n1 = divmod(i0 + 1, N)
                jointT0 = joint_p.tile([H, AC, P], bf16, tag="joint")
                jointT1 = joint_p.tile([H, AC, P], bf16, tag="joint")
                masked2 = small_p.tile([P, 2, AC], f32, tag="mask")

                # previous group's exp/reduce first: precise deps on its own
                # masks (done last period) keep the ACT queue from stalling
                if g >= 1:
                    e2p = exp_reduce(g - 1, pend_ab[g - 1][1])
                ja0, jb0 = joint_mm(i0, b0, objW, att1_c, 0)
                relu_act(i0, bias_all, ja0, jointT0)
                relu_dve(i0, bias_all, jb0, jointT0)
                ja1, jb1 = joint_mm(i0 + 1, b1_, objW, att1_c, 1)
                relu_act(i0 + 1, bias_all, ja1, jointT1)
                relu_dve(i0 + 1, bias_all, jb1, jointT1)
                if g >= 1:
                    final_mm(g - 1, pend_ab.pop(g - 1)[0], e2p)
                l0 = logits_mm(jointT0, w2_b)
                mask_add(b0, n0, negm, l0, masked2, 0)
                l1 = logits_mm(jointT1, w2_b)
                mask_add(b1_, n1, negm, l1, masked2, 1)
                pend_ab[g] = (att1_c, masked2)

            g = NG - 1
            e2p = exp_reduce(g, pend_ab[g][1])
            final_mm(g, pend_ab.pop(g)[0], e2p)

            nc.vector.tensor_copy(outbuf[:, 0:NP], ps_out)
            nc.sync.dma_start(outs_d[:], outbuf)

    nc.compile()
    return nc


def _get_nc(AC=None):
    global _NC_LAST
    if AC is None:
        if _NC_LAST is not None:
            return _NC_LAST
        AC = 5
    if AC not in _NC_CACHE:
        _NC_CACHE[AC] = _build_nc(AC)
    _NC_LAST = _NC_CACHE[AC]
    return _NC_LAST


def kernel(**inputs):
    q = np.asarray(inputs["q"], dtype=np.float32)
    att1 = np.asarray(inputs["att1"], dtype=np.float32)
    obj = np.asarray(inputs["obj_reps"], dtype=np.float32)
    tags = np.asarray(inputs["tags_attention"], dtype=np.int32)
    W1 = np.asarray(inputs["W1"], dtype=np.float32)
    b1 = np.asarray(inputs["b1"], dtype=np.float32)
    W2 = np.asarray(inputs["W2"], dtype=np.float32)
    t = float(np.asarray(inputs["t"]))
    # b2 dropped: constant shift is softmax-invariant.

    import ml_dtypes

    # ---- sparsity compaction: keep only tag==1 rows of att1 ----
    cnt = tags.sum(axis=-1)                      # [B, N]
    AC = max(2, int(-(-int(cnt.max()) // P)))    # chunks of 128
    ACU = AC * P
    order = np.argsort(1 - tags, axis=-1, kind="stable")[..., :ACU]  # [B,N,ACU]
    att1_comp = np.take_along_axis(att1, order[..., None], axis=2)   # [B,N,ACU,O]
    valid = np.take_along_axis(tags, order, axis=2)                  # [B,N,ACU]
    negm_full = (valid.astype(np.float32) - 1.0) * 1e30              # 0 / -1e30

    att1_bf = att1_comp.astype(ml_dtypes.bfloat16)
    nat = att1_bf.reshape(B, N, AC, P, O).transpose(0, 1, 3, 2, 4) \
        .reshape(B, N, P, ACU)
    trans = att1_bf.transpose(0, 1, 3, 2)                            # [B,N,O,ACU]
    att1_c = np.concatenate([nat, trans], axis=-1)                   # [B,N,128,2ACU]

    nc = _get_nc(AC)
    from concourse.bass_utils import run_bass_kernel_spmd

    w1r = np.ascontiguousarray(
        W1.reshape(4, P, H).transpose(1, 0, 2)).astype(ml_dtypes.bfloat16)
    w2s = np.ascontiguousarray((W2 / t).reshape(H, 1)).astype(ml_dtypes.bfloat16)
    b1r = np.ascontiguousarray(b1.reshape(H, 1))

    in_maps = []
    for k in range(NCORES):
        bs = slice(k * BPC, (k + 1) * BPC)
        q_t = q[bs].reshape(BPC * N, 2, P).transpose(1, 2, 0) \
            .astype(ml_dtypes.bfloat16)                              # [2,P,NP]
        objT_t = obj[bs].transpose(2, 0, 1).reshape(2, P, BPC, O) \
            .astype(ml_dtypes.bfloat16)                              # [2,P,BPC,O]
        negm_t = negm_full[bs].reshape(BPC, N, AC, P).transpose(3, 0, 1, 2)
        in_maps.append({
            "att1": np.ascontiguousarray(att1_c[bs]),
            "obj": np.ascontiguousarray(objT_t),
            "q": np.ascontiguousarray(q_t),
            "negm": np.ascontiguousarray(negm_t),
            "w1": w1r,
            "w2": w2s,
            "b1": b1r,
        })

    res = run_bass_kernel_spmd(nc, in_maps, core_ids=list(range(NCORES)),
                               trace=TRACE, **TRACE_KW)
    outs = []
    for r in res.results:
        raw = r["outs"]                          # [P, 2*NP] f32
        cols = raw[:, :NP]                       # [o, pair]
        s = raw[:, NP:].sum(axis=0)              # [pair]
        outs.append((cols / s[None, :]).T.reshape(BPC, N, O))
    out = np.concatenate(outs, axis=0)
    if TRACE:
        print("HW exec time:", res.exec_time_ns, "ns",
              "(mean:", res.mean_exec_time_ns, ")")
        if res.instructions_and_trace:
            print("trace:", res.instructions_and_trace[1])
    return out
